# revision 10
# baseline (speedup 1.0000x reference)
"""Trainium2 Bass kernel for nn_Attn_88364657148092.

GQA causal attention (B=2, T=2048, D=1024, NH=16, NKV=4, HD=64) with
qk rms-norm + RoPE + per-head q gain, value-residual, sigmoid-gated
output projection. Sharded over 8 NeuronCores: data-parallel over batch
x tensor-parallel over kv-head groups (GQA groups intact). Each core
computes a full [T, D] partial of the output projection (ow sharded on
its input dim); the host sums the 4 partials per batch.

Self-contained: hardcodes all shapes; imports the Bass toolchain from
/opt/trn_rl_repo.
"""
import sys

sys.path.insert(0, "/opt/trn_rl_repo")

import numpy as np  # noqa: E402

import concourse.bacc as bacc  # noqa: E402
import concourse.mybir as mybir  # noqa: E402
import concourse.tile as tile  # noqa: E402
from concourse.bass_utils import run_bass_kernel_spmd  # noqa: E402

B, T, D = 2, 2048, 1024
NH, NKV, HD = 16, 4, 64
G = NH // NKV          # 4 q-heads per kv head (= per core)
H2 = HD // 2           # 32
BASE, TSL = 10000.0, 1024
EPS = 1.1920929e-07
NEG = -1.0e30
NT = T // 512          # 4 q-chunks of 512
NKB = T // 128         # 16 k-blocks of 128
DC = D // 128          # 8 contraction chunks

F32 = mybir.dt.float32
F32R = mybir.dt.float32r
AF = mybir.ActivationFunctionType
OP = mybir.AluOpType

_CACHE = {}


def _build_program(lam1: float, reps: int = 1):
    nc = bacc.Bacc("TRN2", target_bir_lowering=False, debug=False, num_devices=8)

    dram = {}
    for name, shape, dt in [
        ("xT", [128, DC, T], F32R), ("qwU", [128, DC, 128], F32R),
        ("qwL", [128, DC, 128], F32R), ("kvw", [128, DC, 128], F32R),
        ("gw", [128, DC, G], F32R), ("ow", [128, 2, D], F32R),
        ("cos4", [128, T], F32), ("sin4", [128, T], F32),
        ("vaux", [128, T], F32), ("trimask", [128, 128], F32),
        ("ident", [64, 64], F32), ("sumq", [128, 5], F32R),
        ("sumk", [64, 5], F32R), ("bcq", [5, 128], F32R),
        ("bck", [5, 64], F32R), ("bchp", [4, 256], F32R),
        ("perm", [64, 64], F32R), ("lnsc", [5, 1], F32),
        ("lnbi", [5, 1], F32), ("gb", [4, 1], F32),
        ("vext0", [128, NKB, 65], F32R),
    ]:
        dram[name] = nc.dram_tensor(name, shape, dt, kind="ExternalInput")
    dram["out"] = nc.dram_tensor("out_p", [128, 16, D], F32, kind="ExternalOutput")
    dram["rawv"] = nc.dram_tensor("raw_v", [64, T], F32, kind="ExternalOutput")

    with tile.TileContext(nc) as tc:
        with tc.tile_pool(name="persist", bufs=1) as pp:
            ct = {}
            for name, shape, dt in [
                ("trimask", [128, 128], F32), ("sumq", [128, 5], F32R),
                ("sumk", [64, 5], F32R), ("bcq", [5, 128], F32R),
                ("bck", [5, 64], F32R), ("bchp", [4, 256], F32R),
                ("perm", [64, 64], F32R), ("ident", [64, 64], F32),
                ("lnsc", [5, 1], F32), ("lnbi", [5, 1], F32), ("gb", [4, 1], F32),
            ]:
                ct[name] = pp.tile(shape, dt, tag=name, name=name)
                nc.sync.dma_start(ct[name][:], dram[name][:])

            for _rep in range(reps):
                _emit_body(nc, tc, dram, ct, lam1)

    nc.compile()
    return nc


def _emit_body(nc, tc, dram, ct, lam1):
    with tc.tile_pool(name="mid", bufs=1) as pm:
        t_qu = pm.tile([128, T], F32, tag="qu")
        t_ql = pm.tile([128, T], F32, tag="ql")
        t_k = pm.tile([64, T], F32R, tag="k")
        t_v = pm.tile([64, T], F32, tag="v")
        t_gate = pm.tile([4, T], F32, tag="gate")

        # ---------- Phase 1a: projections ----------
        with (
            tc.tile_pool(name="p1aps", bufs=2, space="PSUM") as psa,
            tc.tile_pool(name="p1asb", bufs=1) as p1a,
        ):
            t_xT = p1a.tile([128, DC, T], F32R, tag="xT")
            for kc in range(DC):
                nc.sync.dma_start(t_xT[:, kc, :], dram["xT"][:, kc, :])
            t_qwU = p1a.tile([128, DC, 128], F32R, tag="qwU")
            t_qwL = p1a.tile([128, DC, 128], F32R, tag="qwL")
            t_kvw = p1a.tile([128, DC, 128], F32R, tag="kvw")
            t_gw = p1a.tile([128, DC, G], F32R, tag="gw")
            nc.sync.dma_start(t_qwU[:], dram["qwU"][:])
            nc.sync.dma_start(t_qwL[:], dram["qwL"][:])
            nc.sync.dma_start(t_kvw[:], dram["kvw"][:])
            nc.sync.dma_start(t_gw[:], dram["gw"][:])

            def proj(wt, mcols):
                ps = psa.tile([mcols, T], F32, tag="big")
                for kc in range(DC):
                    for n in range(NT):
                        nc.tensor.matmul(
                            ps[:, 512 * n:512 * n + 512],
                            wt[:, kc, :],
                            t_xT[:, kc, 512 * n:512 * n + 512],
                            start=(kc == 0), stop=(kc == DC - 1))
                return ps

            qu_ps = proj(t_qwU, 128)
            nc.scalar.copy(t_qu[:], qu_ps[:])
            ql_ps = proj(t_qwL, 128)
            nc.scalar.copy(t_ql[:], ql_ps[:])
            kv_ps = proj(t_kvw, 128)
            nc.scalar.copy(t_k[:], kv_ps[0:64, :])
            nc.scalar.copy(t_v[:], kv_ps[64:128, :])
            g_ps = proj(t_gw, G)
            nc.scalar.activation(t_gate[:], g_ps[:], AF.Sigmoid, bias=ct["gb"][:])

        # ---------- Phase 1b: norms, rope, v prep ----------
        with tc.tile_pool(name="at", bufs=1) as pa:
            t_qhu = pa.tile([128, T], F32R, tag="qhu")
            t_qhl = pa.tile([128, T], F32R, tag="qhl")
            t_ku4 = pa.tile([128, T], F32R, tag="ku4")
            t_kl4 = pa.tile([128, T], F32R, tag="kl4")
            t_vext = pa.tile([128, NKB, 65], F32R, tag="vext")
            t_vraw = pa.tile([64, T], F32, tag="vraw")
            t_yt = [pa.tile([128, T], F32R, tag=f"yt{pr}", name=f"yt{pr}") for pr in range(2)]
            t_sums = pa.tile([4, T], F32, tag="sums")
            t_sumh = [pa.tile([1, T], F32, tag=f"sumh{h}", name=f"sumh{h}")
                      for h in range(4)]
            nc.sync.dma_start(t_vext[:], dram["vext0"][:])

            with (
                tc.tile_pool(name="p1bps", bufs=2, space="PSUM") as psb,
                tc.tile_pool(name="p1bsb", bufs=1) as p1b,
            ):
                t_cos4 = p1b.tile([128, T], F32, tag="cos4")
                t_sin4 = p1b.tile([128, T], F32, tag="sin4")
                t_vaux = p1b.tile([64, T], F32, tag="vaux")
                t_v0s = p1b.tile([64, T], F32, tag="v0s")
                nc.sync.dma_start(t_cos4[:], dram["cos4"][:])
                nc.sync.dma_start(t_sin4[:], dram["sin4"][:])
                nc.sync.dma_start(t_vaux[:], dram["vaux"][0:64, :])
                nc.sync.dma_start(t_v0s[:], dram["vaux"][64:128, :])

                def tmp(shape, dt):
                    return p1b.tile(shape, dt, tag="tmp", bufs=3, name="tmp")

                # squares -> ssq -> rstd
                t_squ = tmp([128, T], F32R)
                nc.vector.tensor_tensor(out=t_squ[:], in0=t_qu[:], in1=t_qu[:],
                                        op=OP.mult)
                t_sql = tmp([128, T], F32R)
                nc.vector.tensor_tensor(out=t_sql[:], in0=t_ql[:], in1=t_ql[:],
                                        op=OP.mult)
                t_sqk = tmp([64, T], F32R)
                nc.vector.tensor_tensor(out=t_sqk[:], in0=t_k[:].bitcast(F32),
                                        in1=t_k[:].bitcast(F32), op=OP.mult)
                ssq_ps = psb.tile([5, T], F32, tag="big")
                for n in range(NT):
                    sl = slice(512 * n, 512 * n + 512)
                    nc.tensor.matmul(ssq_ps[:, sl], ct["sumq"][:], t_squ[:, sl],
                                     start=True, stop=False)
                    nc.tensor.matmul(ssq_ps[:, sl], ct["sumq"][:], t_sql[:, sl],
                                     start=False, stop=False)
                    nc.tensor.matmul(ssq_ps[:, sl], ct["sumk"][:], t_sqk[:, sl],
                                     start=False, stop=True)
                t_ln = tmp([5, T], F32)
                nc.scalar.activation(t_ln[:], ssq_ps[:], AF.Ln,
                                     bias=ct["lnbi"][:], scale=ct["lnsc"][:])
                t_rstd = p1b.tile([5, T], F32R, tag="rstd")
                nc.scalar.activation(t_rstd[:], t_ln[:], AF.Exp, scale=-0.5)

                # k rope + norm
                bck_ps = psb.tile([64, T], F32, tag="big")
                for n in range(NT):
                    sl = slice(512 * n, 512 * n + 512)
                    nc.tensor.matmul(bck_ps[:, sl], ct["bck"][:], t_rstd[:, sl],
                                     start=True, stop=True)
                rot_ps = psb.tile([64, T], F32, tag="big")
                for n in range(NT):
                    sl = slice(512 * n, 512 * n + 512)
                    nc.tensor.matmul(rot_ps[:, sl], ct["perm"][:], t_k[:, sl],
                                     start=True, stop=True)
                t_km1 = tmp([64, T], F32)
                nc.vector.tensor_tensor(out=t_km1[:], in0=t_k[:].bitcast(F32),
                                        in1=t_cos4[0:64, :], op=OP.mult)
                t_km2 = tmp([64, T], F32)
                nc.vector.tensor_tensor(out=t_km2[:], in0=rot_ps[:],
                                        in1=t_sin4[0:64, :], op=OP.mult)
                t_kraw = tmp([64, T], F32)
                nc.vector.tensor_tensor(out=t_kraw[:], in0=t_km1[:],
                                        in1=t_km2[:], op=OP.add)
                t_khat = tmp([64, T], F32R)
                nc.vector.tensor_tensor(out=t_khat[:], in0=t_kraw[:],
                                        in1=bck_ps[:], op=OP.mult)
                for r in range(4):
                    nc.sync.dma_start(t_ku4[32 * r:32 * r + 32, :],
                                      t_khat[0:32, :])
                    nc.sync.dma_start(t_kl4[32 * r:32 * r + 32, :],
                                      t_khat[32:64, :])

                # q rope + norm
                bcq_ps = psb.tile([128, T], F32, tag="big")
                for n in range(NT):
                    sl = slice(512 * n, 512 * n + 512)
                    nc.tensor.matmul(bcq_ps[:, sl], ct["bcq"][:], t_rstd[:, sl],
                                     start=True, stop=True)
                t_a = tmp([128, T], F32)
                nc.vector.tensor_tensor(out=t_a[:], in0=t_qu[:], in1=t_cos4[:],
                                        op=OP.mult)
                t_b = tmp([128, T], F32)
                nc.vector.tensor_tensor(out=t_b[:], in0=t_ql[:], in1=t_sin4[:],
                                        op=OP.mult)
                t_ab = tmp([128, T], F32)
                nc.vector.tensor_tensor(out=t_ab[:], in0=t_a[:], in1=t_b[:],
                                        op=OP.add)
                nc.vector.tensor_tensor(out=t_qhu[:], in0=t_ab[:],
                                        in1=bcq_ps[:], op=OP.mult)
                t_a2 = tmp([128, T], F32)
                nc.vector.tensor_tensor(out=t_a2[:], in0=t_ql[:], in1=t_cos4[:],
                                        op=OP.mult)
                t_b2 = tmp([128, T], F32)
                nc.vector.tensor_tensor(out=t_b2[:], in0=t_qu[:], in1=t_sin4[:],
                                        op=OP.mult)
                t_ab2 = tmp([128, T], F32)
                nc.vector.tensor_tensor(out=t_ab2[:], in0=t_a2[:], in1=t_b2[:],
                                        op=OP.subtract)
                nc.vector.tensor_tensor(out=t_qhl[:], in0=t_ab2[:],
                                        in1=bcq_ps[:], op=OP.mult)

                # v: raw + residual mix; transpose to [k, hd] with ones col
                nc.vector.tensor_tensor(out=t_vraw[:], in0=t_v[:],
                                        in1=t_vaux[:], op=OP.add)
                nc.sync.dma_start(dram["rawv"][:], t_vraw[:])
                t_vfin = tmp([64, T], F32)
                nc.vector.scalar_tensor_tensor(
                    out=t_vfin[:], in0=t_vraw[:], scalar=lam1,
                    in1=t_v0s[:], op0=OP.mult, op1=OP.add)
                for kb in range(NKB):
                    tr_ps = psb.tile([128, 64], F32, tag="big")
                    nc.tensor.transpose(tr_ps[:],
                                        t_vfin[:, 128 * kb:128 * kb + 128],
                                        ct["ident"][:])
                    nc.vector.tensor_copy(t_vext[:, kb, 0:64], tr_ps[:])

            # ---------- Phase 2: attention ----------
            with (
                tc.tile_pool(name="p2ps", bufs=1, space="PSUM") as ps2,
                tc.tile_pool(name="p2sb", bufs=1) as p2,
            ):
                for j in range(NT):
                    qlo = 512 * j
                    pv = [ps2.tile([65, 512], F32, tag=f"pv{h}", name=f"pv{h}_{j}")
                          for h in range(4)]
                    for kb in range(4 * j + 4):
                        diag = kb >= 4 * j
                        qs = 128 * (kb - 4 * j) if diag else 0
                        ksl = slice(128 * kb, 128 * kb + 128)
                        qsl = slice(qlo + qs, qlo + 512)
                        for pr in range(2):
                            st = ps2.tile([128, 1024], F32, tag="st", bufs=2)
                            for hl in range(2):
                                h = 2 * pr + hl
                                rsl = slice(32 * h, 32 * h + 32)
                                osl = slice(512 * hl + qs, 512 * hl + 512)
                                nc.tensor.matmul(
                                    st[:, osl], t_ku4[rsl, ksl],
                                    t_qhu[rsl, qsl], start=True, stop=False,
                                    tile_position=(32 * h, 0))
                                nc.tensor.matmul(
                                    st[:, osl], t_kl4[rsl, ksl],
                                    t_qhl[rsl, qsl], start=False, stop=True,
                                    tile_position=(32 * h, 0))
                            pt = p2.tile([128, 1024], F32R, tag="pt", bufs=4)
                            if diag:
                                for hl in range(2):
                                    msl = slice(512 * hl + qs,
                                                512 * hl + qs + 128)
                                    nc.vector.tensor_tensor(
                                        out=st[:, msl], in0=st[:, msl],
                                        in1=ct["trimask"][:], op=OP.add)
                                    esl = slice(512 * hl + qs, 512 * hl + 512)
                                    nc.scalar.activation(pt[:, esl],
                                                         st[:, esl], AF.Exp)
                            else:
                                nc.scalar.activation(pt[:], st[:], AF.Exp)
                            for hl in range(2):
                                h = 2 * pr + hl
                                nc.tensor.matmul(
                                    pv[h][:, qs:512], t_vext[:, kb, :],
                                    pt[:, 512 * hl + qs:512 * hl + 512],
                                    start=(kb == 0), stop=(kb == 4 * j + 3))
                    for h in range(4):
                        pr, hl = h // 2, h % 2
                        dst = t_yt[pr][64 * hl:64 * hl + 64, qlo:qlo + 512]
                        if hl == 0:
                            nc.vector.tensor_copy(dst, pv[h][0:64, :])
                        else:
                            nc.scalar.copy(dst, pv[h][0:64, :])
                        nc.vector.tensor_copy(t_sumh[h][0:1, qlo:qlo + 512],
                                              pv[h][64:65, :])

            # ---------- Phase 3: normalize + gate, output projection ----------
            with (
                tc.tile_pool(name="p3ps", bufs=1, space="PSUM") as ps3,
                tc.tile_pool(name="p3sb", bufs=1) as p3,
            ):
                t_ow = p3.tile([128, 2, D], F32R, tag="ow")
                nc.sync.dma_start(t_ow[:], dram["ow"][:])
                for h in range(4):
                    nc.sync.dma_start(t_sums[h:h + 1, :], t_sumh[h][:])
                t_rec = p3.tile([4, T], F32, tag="rec")
                t_scr = p3.tile([4, T], F32, tag="scr")
                nc.vector.reciprocal_approx_accurate(t_rec[:], t_sums[:],
                                                     t_scr[:])
                t_sc = p3.tile([4, T], F32R, tag="sc")
                nc.vector.tensor_tensor(out=t_sc[:], in0=t_rec[:],
                                        in1=t_gate[:], op=OP.mult)
                t_yts = [p3.tile([128, T], F32R, tag=f"yts{pr}", name=f"yts{pr}")
                         for pr in range(2)]
                for pr in range(2):
                    scb_ps = ps3.tile([128, T], F32, tag="scb")
                    for n in range(NT):
                        sl = slice(512 * n, 512 * n + 512)
                        nc.tensor.matmul(scb_ps[:, sl],
                                         ct["bchp"][:, 128 * pr:128 * pr + 128],
                                         t_sc[:, sl], start=True, stop=True)
                    nc.vector.tensor_tensor(out=t_yts[pr][:],
                                            in0=t_yt[pr][:].bitcast(F32),
                                            in1=scb_ps[:], op=OP.mult)
                for tch in range(16):
                    tsl = slice(128 * tch, 128 * tch + 128)
                    o_ps = ps3.tile([128, D], F32, tag="ops", bufs=2)
                    for e2 in range(2):
                        esl = slice(512 * e2, 512 * e2 + 512)
                        for pr in range(2):
                            nc.tensor.matmul(o_ps[:, esl], t_yts[pr][:, tsl],
                                             t_ow[:, pr, esl],
                                             start=(pr == 0), stop=(pr == 1))
                    t_o = p3.tile([128, D], F32, tag="osb", bufs=2)
                    if tch % 2 == 0:
                        nc.vector.tensor_copy(t_o[:], o_ps[:])
                    else:
                        nc.scalar.copy(t_o[:], o_ps[:])
                    nc.sync.dma_start(dram["out"][:, tch, :], t_o[:])


# ======================= host side =======================

def _rope_tables():
    rd = HD
    if T > TSL:
        nb = BASE * (T / TSL) ** (rd / (rd - 2))
    else:
        nb = BASE
    inv = 1.0 / nb ** (np.arange(0, rd, 2, dtype=np.float32) / rd)
    fr = np.arange(T, dtype=np.float32)[:, None] * inv[None, :]
    return (np.ascontiguousarray(np.cos(fr).T.astype(np.float32)),
            np.ascontiguousarray(np.sin(fr).T.astype(np.float32)))


def _const_inputs():
    cosT, sinT = _rope_tables()                             # [32, T]
    cos4 = np.tile(cosT, (4, 1))                            # [128, T]
    sin4 = np.tile(sinT, (4, 1))

    tri = np.where(np.arange(128)[:, None] > np.arange(128)[None, :],
                   np.float32(NEG), np.float32(0.0)).astype(np.float32)
    ident = np.eye(64, dtype=np.float32)

    sumq = np.zeros((128, 5), np.float32)
    for g in range(G):
        sumq[32 * g:32 * g + 32, g] = 1.0
    sumk = np.zeros((64, 5), np.float32)
    sumk[:, 4] = 1.0

    bcq = np.zeros((5, 128), np.float32)
    for m_ in range(128):
        bcq[m_ // 32, m_] = 1.0
    bck = np.zeros((5, 64), np.float32)
    bck[4, :] = 1.0
    bchp = np.zeros((4, 256), np.float32)
    for pr in range(2):
        for m_ in range(128):
            bchp[2 * pr + m_ // 64, 128 * pr + m_] = 1.0

    perm = np.zeros((64, 64), np.float32)
    for m_ in range(H2):
        perm[H2 + m_, m_] = 1.0
    for m_ in range(H2, HD):
        perm[m_ - H2, m_] = -1.0

    vext0 = np.zeros((128, NKB, 65), np.float32)
    vext0[:, :, 64] = 1.0

    return dict(cos4=cos4, sin4=sin4, trimask=tri, ident=ident, sumq=sumq,
                sumk=sumk, bcq=bcq, bck=bck, bchp=bchp, perm=perm, vext0=vext0)


def _prep_core(c, x, qw, kw, vw, ow, ve, v0, q_gain, vr_lambda, gate_w, gate_b,
               const):
    b, h = c // NKV, c % NKV
    lam0 = float(vr_lambda[0])

    xT = np.ascontiguousarray(x[b].T)                       # [D, T]
    xT = xT.reshape(DC, 128, T).transpose(1, 0, 2)          # [128, DC, T]

    qwh = qw.reshape(NH, HD, D)[G * h:G * h + G]            # [4, 64, D]
    qU = qwh[:, 0:H2, :].reshape(128, D)
    qL = qwh[:, H2:HD, :].reshape(128, D)
    qwU = qU.T.reshape(DC, 128, 128).transpose(1, 0, 2)
    qwL = qL.T.reshape(DC, 128, 128).transpose(1, 0, 2)

    kwh = kw.reshape(NKV, HD, D)[h]                         # [64, D]
    vwh = vw.reshape(NKV, HD, D)[h]
    kvr = np.concatenate([kwh[0:H2], kwh[H2:HD], vwh], axis=0)   # [128, D]
    kvw_ = kvr.T.reshape(DC, 128, 128).transpose(1, 0, 2)

    gwh = gate_w[G * h:G * h + G]                           # [4, D]
    gwt = gwh.T.reshape(DC, 128, G).transpose(1, 0, 2)

    ows = ow.T[256 * h:256 * h + 256, :]                    # [256, D]
    owt = ows.reshape(2, 128, D).transpose(1, 0, 2)

    veT = ve[b, :, HD * h:HD * h + HD].T                    # [64, T]
    v0T = v0[b, :, h, :].T
    vaux = np.concatenate([veT, lam0 * v0T], axis=0)        # [128, T]

    gains = q_gain[G * h:G * h + G].astype(np.float64)
    lnsc = np.zeros((5, 1), np.float32)
    lnbi = np.zeros((5, 1), np.float32)
    for g in range(G):
        f = (8.0 / gains[g]) ** 2
        lnsc[g, 0] = f / HD
        lnbi[g, 0] = EPS * f
    lnsc[4, 0] = 1.0 / HD
    lnbi[4, 0] = EPS

    gb = gate_b[G * h:G * h + G].astype(np.float32).reshape(4, 1)

    m = dict(xT=xT, qwU=qwU, qwL=qwL, kvw=kvw_, gw=gwt, ow=owt,
             vaux=vaux, lnsc=lnsc, lnbi=lnbi, gb=gb)
    m.update(const)
    return {k: np.ascontiguousarray(v) for k, v in m.items()}


def kernel(x, qw, kw, vw, ow, ve, v0, q_gain, vr_lambda, gate_w, gate_b,
           _reps=1, _return_res=False):
    x = np.asarray(x, np.float32)
    qw = np.asarray(qw, np.float32)
    kw = np.asarray(kw, np.float32)
    vw = np.asarray(vw, np.float32)
    ow = np.asarray(ow, np.float32)
    ve = np.asarray(ve, np.float32)
    v0 = np.asarray(v0, np.float32)
    q_gain = np.asarray(q_gain, np.float32)
    vr_lambda = np.asarray(vr_lambda, np.float32)
    gate_w = np.asarray(gate_w, np.float32)
    gate_b = np.asarray(gate_b, np.float32)

    lam1 = float(vr_lambda[1])
    key = (lam1, _reps)
    if key not in _CACHE:
        _CACHE[key] = _build_program(lam1, _reps)
    nc = _CACHE[key]

    const = _const_inputs()
    in_maps = [
        _prep_core(c, x, qw, kw, vw, ow, ve, v0, q_gain, vr_lambda,
                   gate_w, gate_b, const)
        for c in range(8)
    ]
    res = run_bass_kernel_spmd(nc, in_maps, core_ids=list(range(8)))

    out = np.zeros((B, T, D), np.float32)
    raw_v = np.zeros((B, T, NKV, HD), np.float32)
    for c in range(8):
        b, h = c // NKV, c % NKV
        po = res.results[c]["out_p"]                        # [128, 16, D]
        out[b] += po.transpose(1, 0, 2).reshape(T, D)
        raw_v[b, :, h, :] = res.results[c]["raw_v"].T
    if _return_res:
        return (out, raw_v), res
    return out, raw_v


# revision 14
# speedup vs baseline: 1.0612x; 1.0612x over previous
"""Trainium2 Bass kernel for nn_Attn_88364657148092.

GQA causal attention (B=2, T=2048, D=1024, NH=16, NKV=4, HD=64) with
qk rms-norm + RoPE + per-head q gain, value-residual, sigmoid-gated
output projection. Sharded over 8 NeuronCores: data-parallel over batch
x tensor-parallel over kv-head groups (GQA groups intact). Each core
computes a full [T, D] partial of the output projection (ow sharded on
its input dim); the host sums the 4 partials per batch.

Self-contained: hardcodes all shapes; imports the Bass toolchain from
/opt/trn_rl_repo.
"""
import sys

sys.path.insert(0, "/opt/trn_rl_repo")

import numpy as np  # noqa: E402

import concourse.bacc as bacc  # noqa: E402
import concourse.mybir as mybir  # noqa: E402
import concourse.tile as tile  # noqa: E402
from concourse.bass_utils import run_bass_kernel_spmd  # noqa: E402

B, T, D = 2, 2048, 1024
NH, NKV, HD = 16, 4, 64
G = NH // NKV          # 4 q-heads per kv head (= per core)
H2 = HD // 2           # 32
BASE, TSL = 10000.0, 1024
EPS = 1.1920929e-07
NEG = -1.0e30
NT = T // 512          # 4 q-chunks of 512
NKB = T // 128         # 16 k-blocks of 128
DC = D // 128          # 8 contraction chunks

F32 = mybir.dt.float32
F32R = mybir.dt.float32r
AF = mybir.ActivationFunctionType
OP = mybir.AluOpType

_CACHE = {}


def _build_program(lam1: float, reps: int = 1):
    nc = bacc.Bacc("TRN2", target_bir_lowering=False, debug=False, num_devices=8)

    dram = {}
    for name, shape, dt in [
        ("xT", [128, DC, T], F32R), ("qwU", [128, DC, 128], F32R),
        ("qwL", [128, DC, 128], F32R), ("kvw", [128, DC, 128], F32R),
        ("gw", [128, DC, G], F32R), ("ow", [128, 2, D], F32R),
        ("cos4", [128, T], F32), ("sin4", [128, T], F32),
        ("vaux", [128, T], F32), ("trimask", [128, 128], F32),
        ("ident", [64, 64], F32), ("sumq", [128, 5], F32R),
        ("sumk", [64, 5], F32R), ("bcq", [5, 128], F32R),
        ("bck", [5, 64], F32R), ("bchp", [4, 256], F32R),
        ("perm", [64, 64], F32R), ("lnsc", [5, 1], F32),
        ("lnbi", [5, 1], F32), ("gb", [4, 1], F32),
        ("vext0", [128, NKB, 65], F32R),
    ]:
        dram[name] = nc.dram_tensor(name, shape, dt, kind="ExternalInput")
    dram["out"] = nc.dram_tensor("out_p", [128, 16, D], F32, kind="ExternalOutput")
    dram["rawv"] = nc.dram_tensor("raw_v", [64, T], F32, kind="ExternalOutput")

    with tile.TileContext(nc) as tc:
        with tc.tile_pool(name="persist", bufs=1) as pp:
            ct = {}
            for name, shape, dt in [
                ("trimask", [128, 128], F32), ("sumq", [128, 5], F32R),
                ("sumk", [64, 5], F32R), ("bcq", [5, 128], F32R),
                ("bck", [5, 64], F32R), ("bchp", [4, 256], F32R),
                ("perm", [64, 64], F32R), ("ident", [64, 64], F32),
                ("lnsc", [5, 1], F32), ("lnbi", [5, 1], F32), ("gb", [4, 1], F32),
            ]:
                ct[name] = pp.tile(shape, dt, tag=name, name=name)
                nc.sync.dma_start(ct[name][:], dram[name][:])

            for _rep in range(reps):
                _emit_body(nc, tc, dram, ct, lam1)

    nc.compile()
    return nc


def _emit_body(nc, tc, dram, ct, lam1):
    with tc.tile_pool(name="mid", bufs=1) as pm:
        t_qu = pm.tile([128, T], F32, tag="qu")
        t_ql = pm.tile([128, T], F32, tag="ql")
        t_k = pm.tile([64, T], F32R, tag="k")
        t_v = pm.tile([64, T], F32, tag="v")
        t_gate = pm.tile([4, T], F32, tag="gate")

        # ---------- Phase 1a: projections ----------
        with (
            tc.tile_pool(name="p1aps", bufs=2, space="PSUM") as psa,
            tc.tile_pool(name="p1asb", bufs=1) as p1a,
        ):
            t_xTs = [p1a.tile([128, T], F32R, tag=f"xT{kc}", name=f"xT{kc}")
                     for kc in range(DC)]
            for kc in range(DC):
                eng = nc.sync if kc % 2 == 0 else nc.gpsimd
                eng.dma_start(t_xTs[kc][:], dram["xT"][:, kc, :])
            t_qwU = p1a.tile([128, DC, 128], F32R, tag="qwU")
            t_qwL = p1a.tile([128, DC, 128], F32R, tag="qwL")
            t_kvw = p1a.tile([128, DC, 128], F32R, tag="kvw")
            t_gw = p1a.tile([128, DC, G], F32R, tag="gw")
            nc.sync.dma_start(t_qwU[:], dram["qwU"][:])
            nc.sync.dma_start(t_qwL[:], dram["qwL"][:])
            nc.sync.dma_start(t_kvw[:], dram["kvw"][:])
            nc.sync.dma_start(t_gw[:], dram["gw"][:])

            def proj_pair(wt_a, ma, wt_b, mb):
                ps_a = psa.tile([ma, T], F32, tag="big", name="ps_a")
                ps_b = psa.tile([mb, T], F32, tag="big", name="ps_b")
                for kc in range(DC):
                    for n in range(NT):
                        nsl = slice(512 * n, 512 * n + 512)
                        nc.tensor.matmul(
                            ps_a[:, nsl], wt_a[:, kc, :], t_xTs[kc][:, nsl],
                            start=(kc == 0), stop=(kc == DC - 1))
                        nc.tensor.matmul(
                            ps_b[:, nsl], wt_b[:, kc, :], t_xTs[kc][:, nsl],
                            start=(kc == 0), stop=(kc == DC - 1))
                return ps_a, ps_b

            qu_ps, ql_ps = proj_pair(t_qwU, 128, t_qwL, 128)
            nc.scalar.copy(t_qu[:], qu_ps[:])
            nc.scalar.copy(t_ql[:], ql_ps[:])
            kv_ps, g_ps = proj_pair(t_kvw, 128, t_gw, G)
            nc.scalar.copy(t_k[:], kv_ps[0:64, :])
            nc.scalar.copy(t_v[:], kv_ps[64:128, :])
            nc.scalar.activation(t_gate[:], g_ps[:], AF.Sigmoid, bias=ct["gb"][:])

        # ---------- Phase 1b: norms, rope, v prep ----------
        with tc.tile_pool(name="at", bufs=1) as pa:
            t_qhu = pa.tile([128, T], F32R, tag="qhu")
            t_qhl = pa.tile([128, T], F32R, tag="qhl")
            t_ku4 = pa.tile([128, T], F32R, tag="ku4")
            t_kl4 = pa.tile([128, T], F32R, tag="kl4")
            t_vext = pa.tile([128, NKB, 65], F32R, tag="vext")
            t_vraw = pa.tile([64, T], F32, tag="vraw")
            t_yt = [pa.tile([128, T], F32R, tag=f"yt{pr}", name=f"yt{pr}") for pr in range(2)]
            t_sums = pa.tile([4, T], F32, tag="sums")
            t_sumh = [pa.tile([1, T], F32, tag=f"sumh{h}", name=f"sumh{h}")
                      for h in range(4)]
            nc.sync.dma_start(t_vext[:], dram["vext0"][:])

            with (
                tc.tile_pool(name="p1bps", bufs=2, space="PSUM") as psb,
                tc.tile_pool(name="p1bsb", bufs=1) as p1b,
            ):
                t_cos4 = p1b.tile([128, T], F32, tag="cos4")
                t_sin4 = p1b.tile([128, T], F32, tag="sin4")
                t_vaux = p1b.tile([64, T], F32, tag="vaux")
                t_v0s = p1b.tile([64, T], F32, tag="v0s")
                nc.sync.dma_start(t_cos4[:], dram["cos4"][:])
                nc.sync.dma_start(t_sin4[:], dram["sin4"][:])
                nc.sync.dma_start(t_vaux[:], dram["vaux"][0:64, :])
                nc.sync.dma_start(t_v0s[:], dram["vaux"][64:128, :])

                def tmp(shape, dt):
                    return p1b.tile(shape, dt, tag="tmp", bufs=3, name="tmp")

                # squares -> ssq -> rstd
                t_squ = tmp([128, T], F32R)
                nc.vector.tensor_tensor(out=t_squ[:], in0=t_qu[:], in1=t_qu[:],
                                        op=OP.mult)
                t_sql = tmp([128, T], F32R)
                nc.vector.tensor_tensor(out=t_sql[:], in0=t_ql[:], in1=t_ql[:],
                                        op=OP.mult)
                t_sqk = tmp([64, T], F32R)
                nc.vector.tensor_tensor(out=t_sqk[:], in0=t_k[:].bitcast(F32),
                                        in1=t_k[:].bitcast(F32), op=OP.mult)
                ssq_ps = psb.tile([5, T], F32, tag="big")
                for n in range(NT):
                    sl = slice(512 * n, 512 * n + 512)
                    nc.tensor.matmul(ssq_ps[:, sl], ct["sumq"][:], t_squ[:, sl],
                                     start=True, stop=False)
                    nc.tensor.matmul(ssq_ps[:, sl], ct["sumq"][:], t_sql[:, sl],
                                     start=False, stop=False)
                    nc.tensor.matmul(ssq_ps[:, sl], ct["sumk"][:], t_sqk[:, sl],
                                     start=False, stop=True)
                t_ln = tmp([5, T], F32)
                nc.scalar.activation(t_ln[:], ssq_ps[:], AF.Ln,
                                     bias=ct["lnbi"][:], scale=ct["lnsc"][:])
                t_rstd = p1b.tile([5, T], F32R, tag="rstd")
                nc.scalar.activation(t_rstd[:], t_ln[:], AF.Exp, scale=-0.5)

                # k rope + norm
                bck_ps = psb.tile([64, T], F32, tag="big")
                for n in range(NT):
                    sl = slice(512 * n, 512 * n + 512)
                    nc.tensor.matmul(bck_ps[:, sl], ct["bck"][:], t_rstd[:, sl],
                                     start=True, stop=True)
                rot_ps = psb.tile([64, T], F32, tag="big")
                for n in range(NT):
                    sl = slice(512 * n, 512 * n + 512)
                    nc.tensor.matmul(rot_ps[:, sl], ct["perm"][:], t_k[:, sl],
                                     start=True, stop=True)
                t_km1 = tmp([64, T], F32)
                nc.vector.tensor_tensor(out=t_km1[:], in0=t_k[:].bitcast(F32),
                                        in1=t_cos4[0:64, :], op=OP.mult)
                t_km2 = tmp([64, T], F32)
                nc.vector.tensor_tensor(out=t_km2[:], in0=rot_ps[:],
                                        in1=t_sin4[0:64, :], op=OP.mult)
                t_kraw = tmp([64, T], F32)
                nc.vector.tensor_tensor(out=t_kraw[:], in0=t_km1[:],
                                        in1=t_km2[:], op=OP.add)
                t_khat = tmp([64, T], F32R)
                nc.vector.tensor_tensor(out=t_khat[:], in0=t_kraw[:],
                                        in1=bck_ps[:], op=OP.mult)
                for r in range(4):
                    nc.sync.dma_start(t_ku4[32 * r:32 * r + 32, :],
                                      t_khat[0:32, :])
                    nc.sync.dma_start(t_kl4[32 * r:32 * r + 32, :],
                                      t_khat[32:64, :])

                # q rope + norm
                bcq_ps = psb.tile([128, T], F32, tag="big")
                for n in range(NT):
                    sl = slice(512 * n, 512 * n + 512)
                    nc.tensor.matmul(bcq_ps[:, sl], ct["bcq"][:], t_rstd[:, sl],
                                     start=True, stop=True)
                t_a = tmp([128, T], F32)
                nc.vector.tensor_tensor(out=t_a[:], in0=t_qu[:], in1=t_cos4[:],
                                        op=OP.mult)
                t_b = tmp([128, T], F32)
                nc.vector.tensor_tensor(out=t_b[:], in0=t_ql[:], in1=t_sin4[:],
                                        op=OP.mult)
                t_ab = tmp([128, T], F32)
                nc.vector.tensor_tensor(out=t_ab[:], in0=t_a[:], in1=t_b[:],
                                        op=OP.add)
                nc.vector.tensor_tensor(out=t_qhu[:], in0=t_ab[:],
                                        in1=bcq_ps[:], op=OP.mult)
                t_a2 = tmp([128, T], F32)
                nc.vector.tensor_tensor(out=t_a2[:], in0=t_ql[:], in1=t_cos4[:],
                                        op=OP.mult)
                t_b2 = tmp([128, T], F32)
                nc.vector.tensor_tensor(out=t_b2[:], in0=t_qu[:], in1=t_sin4[:],
                                        op=OP.mult)
                t_ab2 = tmp([128, T], F32)
                nc.vector.tensor_tensor(out=t_ab2[:], in0=t_a2[:], in1=t_b2[:],
                                        op=OP.subtract)
                nc.vector.tensor_tensor(out=t_qhl[:], in0=t_ab2[:],
                                        in1=bcq_ps[:], op=OP.mult)

                # v: raw + residual mix; transpose to [k, hd] with ones col
                nc.vector.tensor_tensor(out=t_vraw[:], in0=t_v[:],
                                        in1=t_vaux[:], op=OP.add)
                nc.sync.dma_start(dram["rawv"][:], t_vraw[:])
                t_vfin = tmp([64, T], F32)
                nc.vector.scalar_tensor_tensor(
                    out=t_vfin[:], in0=t_vraw[:], scalar=lam1,
                    in1=t_v0s[:], op0=OP.mult, op1=OP.add)
                for kb in range(NKB):
                    tr_ps = psb.tile([128, 64], F32, tag="big")
                    nc.tensor.transpose(tr_ps[:],
                                        t_vfin[:, 128 * kb:128 * kb + 128],
                                        ct["ident"][:])
                    nc.vector.tensor_copy(t_vext[:, kb, 0:64], tr_ps[:])

            # ---------- Phase 2: attention ----------
            with (
                tc.tile_pool(name="p2ps", bufs=1, space="PSUM") as ps2,
                tc.tile_pool(name="p2sb", bufs=1) as p2,
            ):
                for j in range(NT):
                    qlo = 512 * j
                    pv = [ps2.tile([65, 512], F32, tag=f"pv{h}", name=f"pv{h}_{j}")
                          for h in range(4)]
                    pending = []

                    def flush_pv():
                        while pending:
                            kb_, pt_, qs_ = pending.pop(0)
                            for hl_ in range(2):
                                for pr_ in range(2):
                                    h_ = 2 * pr_ + hl_
                                    nc.tensor.matmul(
                                        pv[h_][:, qs_:512], t_vext[:, kb_, :],
                                        pt_[pr_][:, 512 * hl_ + qs_:
                                                 512 * hl_ + 512],
                                        start=(kb_ == 0),
                                        stop=(kb_ == 4 * j + 3))

                    for kb in range(4 * j + 4):
                        diag = kb >= 4 * j
                        qs = 128 * (kb - 4 * j) if diag else 0
                        ksl = slice(128 * kb, 128 * kb + 128)
                        qsl = slice(qlo + qs, qlo + 512)
                        pts = []
                        for pr in range(2):
                            st = ps2.tile([128, 1024], F32, tag="st", bufs=2)
                            for hl in range(2):
                                h = 2 * pr + hl
                                rsl = slice(32 * h, 32 * h + 32)
                                osl = slice(512 * hl + qs, 512 * hl + 512)
                                nc.tensor.matmul(
                                    st[:, osl], t_ku4[rsl, ksl],
                                    t_qhu[rsl, qsl], start=True, stop=False,
                                    tile_position=(32 * h, 0))
                                nc.tensor.matmul(
                                    st[:, osl], t_kl4[rsl, ksl],
                                    t_qhl[rsl, qsl], start=False, stop=True,
                                    tile_position=(32 * h, 0))
                            pt = p2.tile([128, 1024], F32R, tag="pt", bufs=6)
                            if diag:
                                for hl in range(2):
                                    msl = slice(512 * hl + qs,
                                                512 * hl + qs + 128)
                                    nc.vector.tensor_tensor(
                                        out=st[:, msl], in0=st[:, msl],
                                        in1=ct["trimask"][:], op=OP.add)
                                    esl = slice(512 * hl + qs, 512 * hl + 512)
                                    nc.scalar.activation(pt[:, esl],
                                                         st[:, esl], AF.Exp)
                            else:
                                nc.scalar.activation(pt[:], st[:], AF.Exp)
                            pts.append(pt)
                        # PV for the PREVIOUS kb is emitted after this kb's QK,
                        # so PE has ready matmuls while ACT runs this exp.
                        if len(pending) >= 2:
                            kb_, pt_, qs_ = pending.pop(0)
                            for hl_ in range(2):
                                for pr_ in range(2):
                                    h_ = 2 * pr_ + hl_
                                    nc.tensor.matmul(
                                        pv[h_][:, qs_:512], t_vext[:, kb_, :],
                                        pt_[pr_][:, 512 * hl_ + qs_:
                                                 512 * hl_ + 512],
                                        start=(kb_ == 0),
                                        stop=(kb_ == 4 * j + 3))
                        pending.append((kb, pts, qs))
                    flush_pv()
                    for h in range(4):
                        pr, hl = h // 2, h % 2
                        dst = t_yt[pr][64 * hl:64 * hl + 64, qlo:qlo + 512]
                        if hl == 0:
                            nc.vector.tensor_copy(dst, pv[h][0:64, :])
                        else:
                            nc.scalar.copy(dst, pv[h][0:64, :])
                        nc.vector.tensor_copy(t_sumh[h][0:1, qlo:qlo + 512],
                                              pv[h][64:65, :])

            # ---------- Phase 3: normalize + gate, output projection ----------
            with (
                tc.tile_pool(name="p3ps", bufs=1, space="PSUM") as ps3,
                tc.tile_pool(name="p3sb", bufs=1) as p3,
            ):
                t_ow = p3.tile([128, 2, D], F32R, tag="ow")
                nc.sync.dma_start(t_ow[:], dram["ow"][:])
                for h in range(4):
                    nc.sync.dma_start(t_sums[h:h + 1, :], t_sumh[h][:])
                t_rec = p3.tile([4, T], F32, tag="rec")
                t_scr = p3.tile([4, T], F32, tag="scr")
                nc.vector.reciprocal_approx_accurate(t_rec[:], t_sums[:],
                                                     t_scr[:])
                t_sc = p3.tile([4, T], F32R, tag="sc")
                nc.vector.tensor_tensor(out=t_sc[:], in0=t_rec[:],
                                        in1=t_gate[:], op=OP.mult)
                t_yts = [p3.tile([128, T], F32R, tag=f"yts{pr}", name=f"yts{pr}")
                         for pr in range(2)]
                for pr in range(2):
                    scb_ps = ps3.tile([128, T], F32, tag="scb")
                    for n in range(NT):
                        sl = slice(512 * n, 512 * n + 512)
                        nc.tensor.matmul(scb_ps[:, sl],
                                         ct["bchp"][:, 128 * pr:128 * pr + 128],
                                         t_sc[:, sl], start=True, stop=True)
                    nc.vector.tensor_tensor(out=t_yts[pr][:],
                                            in0=t_yt[pr][:].bitcast(F32),
                                            in1=scb_ps[:], op=OP.mult)
                for tch in range(16):
                    tsl = slice(128 * tch, 128 * tch + 128)
                    o_ps = ps3.tile([128, D], F32, tag="ops", bufs=2)
                    for e2 in range(2):
                        esl = slice(512 * e2, 512 * e2 + 512)
                        for pr in range(2):
                            nc.tensor.matmul(o_ps[:, esl], t_yts[pr][:, tsl],
                                             t_ow[:, pr, esl],
                                             start=(pr == 0), stop=(pr == 1))
                    t_o = p3.tile([128, D], F32, tag="osb", bufs=2)
                    if tch % 2 == 0:
                        nc.vector.tensor_copy(t_o[:], o_ps[:])
                    else:
                        nc.scalar.copy(t_o[:], o_ps[:])
                    eng = nc.sync if tch % 2 == 0 else nc.gpsimd
                    eng.dma_start(dram["out"][:, tch, :], t_o[:])


# ======================= host side =======================

def _rope_tables():
    rd = HD
    if T > TSL:
        nb = BASE * (T / TSL) ** (rd / (rd - 2))
    else:
        nb = BASE
    inv = 1.0 / nb ** (np.arange(0, rd, 2, dtype=np.float32) / rd)
    fr = np.arange(T, dtype=np.float32)[:, None] * inv[None, :]
    return (np.ascontiguousarray(np.cos(fr).T.astype(np.float32)),
            np.ascontiguousarray(np.sin(fr).T.astype(np.float32)))


def _const_inputs():
    cosT, sinT = _rope_tables()                             # [32, T]
    cos4 = np.tile(cosT, (4, 1))                            # [128, T]
    sin4 = np.tile(sinT, (4, 1))

    tri = np.where(np.arange(128)[:, None] > np.arange(128)[None, :],
                   np.float32(NEG), np.float32(0.0)).astype(np.float32)
    ident = np.eye(64, dtype=np.float32)

    sumq = np.zeros((128, 5), np.float32)
    for g in range(G):
        sumq[32 * g:32 * g + 32, g] = 1.0
    sumk = np.zeros((64, 5), np.float32)
    sumk[:, 4] = 1.0

    bcq = np.zeros((5, 128), np.float32)
    for m_ in range(128):
        bcq[m_ // 32, m_] = 1.0
    bck = np.zeros((5, 64), np.float32)
    bck[4, :] = 1.0
    bchp = np.zeros((4, 256), np.float32)
    for pr in range(2):
        for m_ in range(128):
            bchp[2 * pr + m_ // 64, 128 * pr + m_] = 1.0

    perm = np.zeros((64, 64), np.float32)
    for m_ in range(H2):
        perm[H2 + m_, m_] = 1.0
    for m_ in range(H2, HD):
        perm[m_ - H2, m_] = -1.0

    vext0 = np.zeros((128, NKB, 65), np.float32)
    vext0[:, :, 64] = 1.0

    return dict(cos4=cos4, sin4=sin4, trimask=tri, ident=ident, sumq=sumq,
                sumk=sumk, bcq=bcq, bck=bck, bchp=bchp, perm=perm, vext0=vext0)


def _prep_core(c, x, qw, kw, vw, ow, ve, v0, q_gain, vr_lambda, gate_w, gate_b,
               const):
    b, h = c // NKV, c % NKV
    lam0 = float(vr_lambda[0])

    xT = np.ascontiguousarray(x[b].T)                       # [D, T]
    xT = xT.reshape(DC, 128, T).transpose(1, 0, 2)          # [128, DC, T]

    qwh = qw.reshape(NH, HD, D)[G * h:G * h + G]            # [4, 64, D]
    qU = qwh[:, 0:H2, :].reshape(128, D)
    qL = qwh[:, H2:HD, :].reshape(128, D)
    qwU = qU.T.reshape(DC, 128, 128).transpose(1, 0, 2)
    qwL = qL.T.reshape(DC, 128, 128).transpose(1, 0, 2)

    kwh = kw.reshape(NKV, HD, D)[h]                         # [64, D]
    vwh = vw.reshape(NKV, HD, D)[h]
    kvr = np.concatenate([kwh[0:H2], kwh[H2:HD], vwh], axis=0)   # [128, D]
    kvw_ = kvr.T.reshape(DC, 128, 128).transpose(1, 0, 2)

    gwh = gate_w[G * h:G * h + G]                           # [4, D]
    gwt = gwh.T.reshape(DC, 128, G).transpose(1, 0, 2)

    ows = ow.T[256 * h:256 * h + 256, :]                    # [256, D]
    owt = ows.reshape(2, 128, D).transpose(1, 0, 2)

    veT = ve[b, :, HD * h:HD * h + HD].T                    # [64, T]
    v0T = v0[b, :, h, :].T
    vaux = np.concatenate([veT, lam0 * v0T], axis=0)        # [128, T]

    gains = q_gain[G * h:G * h + G].astype(np.float64)
    lnsc = np.zeros((5, 1), np.float32)
    lnbi = np.zeros((5, 1), np.float32)
    for g in range(G):
        f = (8.0 / gains[g]) ** 2
        lnsc[g, 0] = f / HD
        lnbi[g, 0] = EPS * f
    lnsc[4, 0] = 1.0 / HD
    lnbi[4, 0] = EPS

    gb = gate_b[G * h:G * h + G].astype(np.float32).reshape(4, 1)

    m = dict(xT=xT, qwU=qwU, qwL=qwL, kvw=kvw_, gw=gwt, ow=owt,
             vaux=vaux, lnsc=lnsc, lnbi=lnbi, gb=gb)
    m.update(const)
    return {k: np.ascontiguousarray(v) for k, v in m.items()}


def kernel(x, qw, kw, vw, ow, ve, v0, q_gain, vr_lambda, gate_w, gate_b,
           _reps=1, _return_res=False):
    x = np.asarray(x, np.float32)
    qw = np.asarray(qw, np.float32)
    kw = np.asarray(kw, np.float32)
    vw = np.asarray(vw, np.float32)
    ow = np.asarray(ow, np.float32)
    ve = np.asarray(ve, np.float32)
    v0 = np.asarray(v0, np.float32)
    q_gain = np.asarray(q_gain, np.float32)
    vr_lambda = np.asarray(vr_lambda, np.float32)
    gate_w = np.asarray(gate_w, np.float32)
    gate_b = np.asarray(gate_b, np.float32)

    lam1 = float(vr_lambda[1])
    key = (lam1, _reps)
    if key not in _CACHE:
        _CACHE[key] = _build_program(lam1, _reps)
    nc = _CACHE[key]

    const = _const_inputs()
    in_maps = [
        _prep_core(c, x, qw, kw, vw, ow, ve, v0, q_gain, vr_lambda,
                   gate_w, gate_b, const)
        for c in range(8)
    ]
    res = run_bass_kernel_spmd(nc, in_maps, core_ids=list(range(8)))

    out = np.zeros((B, T, D), np.float32)
    raw_v = np.zeros((B, T, NKV, HD), np.float32)
    for c in range(8):
        b, h = c // NKV, c % NKV
        po = res.results[c]["out_p"]                        # [128, 16, D]
        out[b] += po.transpose(1, 0, 2).reshape(T, D)
        raw_v[b, :, h, :] = res.results[c]["raw_v"].T
    if _return_res:
        return (out, raw_v), res
    return out, raw_v


# revision 19
# speedup vs baseline: 1.0791x; 1.0169x over previous
"""Trainium2 Bass kernel for nn_Attn_88364657148092.

GQA causal attention (B=2, T=2048, D=1024, NH=16, NKV=4, HD=64) with
qk rms-norm + RoPE + per-head q gain, value-residual, sigmoid-gated
output projection. Sharded over 8 NeuronCores: data-parallel over batch
x tensor-parallel over kv-head groups (GQA groups intact). Each core
computes a full [T, D] partial of the output projection (ow sharded on
its input dim); the host sums the 4 partials per batch.

Self-contained: hardcodes all shapes; imports the Bass toolchain from
/opt/trn_rl_repo.
"""
import sys

sys.path.insert(0, "/opt/trn_rl_repo")

import numpy as np  # noqa: E402

import concourse.bacc as bacc  # noqa: E402
import concourse.mybir as mybir  # noqa: E402
import concourse.tile as tile  # noqa: E402
from concourse.bass_utils import run_bass_kernel_spmd  # noqa: E402

B, T, D = 2, 2048, 1024
NH, NKV, HD = 16, 4, 64
G = NH // NKV          # 4 q-heads per kv head (= per core)
H2 = HD // 2           # 32
BASE, TSL = 10000.0, 1024
EPS = 1.1920929e-07
NEG = -1.0e30
NT = T // 512          # 4 q-chunks of 512
NKB = T // 128         # 16 k-blocks of 128
DC = D // 128          # 8 contraction chunks

F32 = mybir.dt.float32
F32R = mybir.dt.float32r
AF = mybir.ActivationFunctionType
OP = mybir.AluOpType

_CACHE = {}


def _build_program(lam1: float, reps: int = 1):
    nc = bacc.Bacc("TRN2", target_bir_lowering=False, debug=False, num_devices=8)

    dram = {}
    for name, shape, dt in [
        ("xT", [128, DC, T], F32R), ("qwU", [128, DC, 128], F32R),
        ("qwL", [128, DC, 128], F32R), ("kvw", [128, DC, 128], F32R),
        ("gw", [128, DC, G], F32R), ("ow", [128, 2, D], F32R),
        ("cos4", [128, T], F32), ("sin4", [128, T], F32),
        ("vaux", [128, T], F32), ("trimask", [128, 128], F32),
        ("ident", [64, 64], F32), ("sumq", [128, 5], F32R),
        ("sumk", [64, 5], F32R), ("bcq", [5, 128], F32R),
        ("bck", [5, 64], F32R), ("bchp", [4, 256], F32R),
        ("perm", [64, 64], F32R), ("lnsc", [5, 1], F32),
        ("lnbi", [5, 1], F32), ("gb", [4, 1], F32),
        ("vext0", [128, NKB, 65], F32R),
    ]:
        dram[name] = nc.dram_tensor(name, shape, dt, kind="ExternalInput")
    dram["out"] = nc.dram_tensor("out_p", [128, 16, D], F32, kind="ExternalOutput")
    dram["rawv"] = nc.dram_tensor("raw_v", [64, T], F32, kind="ExternalOutput")

    with tile.TileContext(nc) as tc:
        with tc.tile_pool(name="persist", bufs=1) as pp:
            ct = {}
            for name, shape, dt in [
                ("trimask", [128, 128], F32), ("sumq", [128, 5], F32R),
                ("sumk", [64, 5], F32R), ("bcq", [5, 128], F32R),
                ("bck", [5, 64], F32R), ("bchp", [4, 256], F32R),
                ("perm", [64, 64], F32R), ("ident", [64, 64], F32),
                ("lnsc", [5, 1], F32), ("lnbi", [5, 1], F32), ("gb", [4, 1], F32),
            ]:
                ct[name] = pp.tile(shape, dt, tag=name, name=name)
                nc.sync.dma_start(ct[name][:], dram[name][:])

            for _rep in range(reps):
                _emit_body(nc, tc, dram, ct, lam1)

    nc.compile()
    return nc


def _emit_body(nc, tc, dram, ct, lam1):
    with tc.tile_pool(name="mid", bufs=1) as pm:
        t_qu = pm.tile([128, T], F32, tag="qu")
        t_ql = pm.tile([128, T], F32, tag="ql")
        t_k = pm.tile([64, T], F32R, tag="k")
        t_v = pm.tile([64, T], F32, tag="v")
        t_gate = pm.tile([4, T], F32, tag="gate")

        # ---------- Phase 1a: projections ----------
        with (
            tc.tile_pool(name="p1aps", bufs=2, space="PSUM") as psa,
            tc.tile_pool(name="p1asb", bufs=1) as p1a,
        ):
            t_xTs = [p1a.tile([128, T], F32R, tag=f"xT{kc}", name=f"xT{kc}")
                     for kc in range(DC)]
            for kc in range(DC):
                eng = nc.sync if kc % 2 == 0 else nc.gpsimd
                eng.dma_start(t_xTs[kc][:], dram["xT"][:, kc, :])
            t_qwU = p1a.tile([128, DC, 128], F32R, tag="qwU")
            t_qwL = p1a.tile([128, DC, 128], F32R, tag="qwL")
            t_kvw = p1a.tile([128, DC, 128], F32R, tag="kvw")
            t_gw = p1a.tile([128, DC, G], F32R, tag="gw")
            nc.sync.dma_start(t_qwU[:], dram["qwU"][:])
            nc.sync.dma_start(t_qwL[:], dram["qwL"][:])
            nc.sync.dma_start(t_kvw[:], dram["kvw"][:])
            nc.sync.dma_start(t_gw[:], dram["gw"][:])

            def proj_pair(wt_a, ma, wt_b, mb):
                ps_a = psa.tile([ma, T], F32, tag="big", name="ps_a")
                ps_b = psa.tile([mb, T], F32, tag="big", name="ps_b")
                for kc in range(DC):
                    for n in range(NT):
                        nsl = slice(512 * n, 512 * n + 512)
                        nc.tensor.matmul(
                            ps_a[:, nsl], wt_a[:, kc, :], t_xTs[kc][:, nsl],
                            start=(kc == 0), stop=(kc == DC - 1))
                        nc.tensor.matmul(
                            ps_b[:, nsl], wt_b[:, kc, :], t_xTs[kc][:, nsl],
                            start=(kc == 0), stop=(kc == DC - 1))
                return ps_a, ps_b

            qu_ps, ql_ps = proj_pair(t_qwU, 128, t_qwL, 128)
            nc.scalar.copy(t_qu[:], qu_ps[:])
            nc.scalar.copy(t_ql[:], ql_ps[:])
            kv_ps, g_ps = proj_pair(t_kvw, 128, t_gw, G)
            nc.scalar.copy(t_k[:], kv_ps[0:64, :])
            nc.scalar.copy(t_v[:], kv_ps[64:128, :])
            nc.scalar.activation(t_gate[:], g_ps[:], AF.Sigmoid, bias=ct["gb"][:])

        # ---------- Phase 1b: norms, rope, v prep ----------
        with tc.tile_pool(name="at", bufs=1) as pa:
            t_qhu = pa.tile([128, T], F32R, tag="qhu")
            t_qhl = pa.tile([128, T], F32R, tag="qhl")
            t_ku4 = pa.tile([128, T], F32R, tag="ku4")
            t_kl4 = pa.tile([128, T], F32R, tag="kl4")
            t_vext = pa.tile([128, NKB, 65], F32R, tag="vext")
            t_vraw = pa.tile([64, T], F32, tag="vraw")
            t_yt = [pa.tile([128, T], F32R, tag=f"yt{pr}", name=f"yt{pr}") for pr in range(2)]
            t_sums = pa.tile([4, T], F32, tag="sums")
            t_sumh = [pa.tile([1, T], F32, tag=f"sumh{h}", name=f"sumh{h}")
                      for h in range(4)]
            nc.sync.dma_start(t_vext[:], dram["vext0"][:])

            with (
                tc.tile_pool(name="p1bps", bufs=2, space="PSUM") as psb,
                tc.tile_pool(name="p1bsb", bufs=1) as p1b,
            ):
                t_cos4 = p1b.tile([128, T], F32, tag="cos4")
                t_sin4 = p1b.tile([128, T], F32, tag="sin4")
                t_vaux = p1b.tile([64, T], F32, tag="vaux")
                t_v0s = p1b.tile([64, T], F32, tag="v0s")
                nc.sync.dma_start(t_cos4[:], dram["cos4"][:])
                nc.sync.dma_start(t_sin4[:], dram["sin4"][:])
                nc.sync.dma_start(t_vaux[:], dram["vaux"][0:64, :])
                nc.sync.dma_start(t_v0s[:], dram["vaux"][64:128, :])

                def tmp(shape, dt):
                    return p1b.tile(shape, dt, tag="tmp", bufs=3, name="tmp")

                # squares -> ssq -> rstd
                t_squ = tmp([128, T], F32R)
                nc.vector.tensor_tensor(out=t_squ[:], in0=t_qu[:], in1=t_qu[:],
                                        op=OP.mult)
                t_sql = tmp([128, T], F32R)
                nc.vector.tensor_tensor(out=t_sql[:], in0=t_ql[:], in1=t_ql[:],
                                        op=OP.mult)
                t_sqk = tmp([64, T], F32R)
                nc.vector.tensor_tensor(out=t_sqk[:], in0=t_k[:].bitcast(F32),
                                        in1=t_k[:].bitcast(F32), op=OP.mult)
                ssq_ps = psb.tile([5, T], F32, tag="big")
                for n in range(NT):
                    sl = slice(512 * n, 512 * n + 512)
                    nc.tensor.matmul(ssq_ps[:, sl], ct["sumq"][:], t_squ[:, sl],
                                     start=True, stop=False)
                    nc.tensor.matmul(ssq_ps[:, sl], ct["sumq"][:], t_sql[:, sl],
                                     start=False, stop=False)
                    nc.tensor.matmul(ssq_ps[:, sl], ct["sumk"][:], t_sqk[:, sl],
                                     start=False, stop=True)
                t_ln = tmp([5, T], F32)
                nc.scalar.activation(t_ln[:], ssq_ps[:], AF.Ln,
                                     bias=ct["lnbi"][:], scale=ct["lnsc"][:])
                t_rstd = p1b.tile([5, T], F32R, tag="rstd")
                nc.scalar.activation(t_rstd[:], t_ln[:], AF.Exp, scale=-0.5)

                # k rope + norm
                bck_ps = psb.tile([64, T], F32, tag="big")
                for n in range(NT):
                    sl = slice(512 * n, 512 * n + 512)
                    nc.tensor.matmul(bck_ps[:, sl], ct["bck"][:], t_rstd[:, sl],
                                     start=True, stop=True)
                rot_ps = psb.tile([64, T], F32, tag="big")
                for n in range(NT):
                    sl = slice(512 * n, 512 * n + 512)
                    nc.tensor.matmul(rot_ps[:, sl], ct["perm"][:], t_k[:, sl],
                                     start=True, stop=True)
                t_km1 = tmp([64, T], F32)
                nc.vector.tensor_tensor(out=t_km1[:], in0=t_k[:].bitcast(F32),
                                        in1=t_cos4[0:64, :], op=OP.mult)
                t_km2 = tmp([64, T], F32)
                nc.vector.tensor_tensor(out=t_km2[:], in0=rot_ps[:],
                                        in1=t_sin4[0:64, :], op=OP.mult)
                t_kraw = tmp([64, T], F32)
                nc.vector.tensor_tensor(out=t_kraw[:], in0=t_km1[:],
                                        in1=t_km2[:], op=OP.add)
                t_khat = tmp([64, T], F32R)
                nc.vector.tensor_tensor(out=t_khat[:], in0=t_kraw[:],
                                        in1=bck_ps[:], op=OP.mult)
                for r in range(4):
                    nc.sync.dma_start(t_ku4[32 * r:32 * r + 32, :],
                                      t_khat[0:32, :])
                    nc.sync.dma_start(t_kl4[32 * r:32 * r + 32, :],
                                      t_khat[32:64, :])

                # v: raw + residual mix; transpose to [k, hd] with ones col
                nc.vector.tensor_tensor(out=t_vraw[:], in0=t_v[:],
                                        in1=t_vaux[:], op=OP.add)
                nc.sync.dma_start(dram["rawv"][:], t_vraw[:])
                t_vfin = tmp([64, T], F32)
                nc.vector.scalar_tensor_tensor(
                    out=t_vfin[:], in0=t_vraw[:], scalar=lam1,
                    in1=t_v0s[:], op0=OP.mult, op1=OP.add)
                for kb in range(NKB):
                    tr_ps = psb.tile([128, 64], F32, tag="big")
                    nc.tensor.transpose(tr_ps[:],
                                        t_vfin[:, 128 * kb:128 * kb + 128],
                                        ct["ident"][:])
                    nc.vector.tensor_copy(t_vext[:, kb, 0:64], tr_ps[:])

                # q rope + norm
                bcq_ps = psb.tile([128, T], F32, tag="big")
                for n in range(NT):
                    sl = slice(512 * n, 512 * n + 512)
                    nc.tensor.matmul(bcq_ps[:, sl], ct["bcq"][:], t_rstd[:, sl],
                                     start=True, stop=True)
                t_a = tmp([128, T], F32)
                nc.vector.tensor_tensor(out=t_a[:], in0=t_qu[:], in1=t_cos4[:],
                                        op=OP.mult)
                t_b = tmp([128, T], F32)
                nc.vector.tensor_tensor(out=t_b[:], in0=t_ql[:], in1=t_sin4[:],
                                        op=OP.mult)
                t_ab = tmp([128, T], F32)
                nc.vector.tensor_tensor(out=t_ab[:], in0=t_a[:], in1=t_b[:],
                                        op=OP.add)
                nc.vector.tensor_tensor(out=t_qhu[:], in0=t_ab[:],
                                        in1=bcq_ps[:], op=OP.mult)
                t_a2 = tmp([128, T], F32)
                nc.vector.tensor_tensor(out=t_a2[:], in0=t_ql[:], in1=t_cos4[:],
                                        op=OP.mult)
                t_b2 = tmp([128, T], F32)
                nc.vector.tensor_tensor(out=t_b2[:], in0=t_qu[:], in1=t_sin4[:],
                                        op=OP.mult)
                t_ab2 = tmp([128, T], F32)
                nc.vector.tensor_tensor(out=t_ab2[:], in0=t_a2[:], in1=t_b2[:],
                                        op=OP.subtract)
                nc.vector.tensor_tensor(out=t_qhl[:], in0=t_ab2[:],
                                        in1=bcq_ps[:], op=OP.mult)

            # ---------- Phase 2: attention ----------
            with (
                tc.tile_pool(name="p2ps", bufs=1, space="PSUM") as ps2,
                tc.tile_pool(name="p2sb", bufs=1) as p2,
            ):
                for j in range(NT):
                    qlo = 512 * j
                    pv = [ps2.tile([65, 512], F32, tag=f"pv{h}", name=f"pv{h}_{j}")
                          for h in range(4)]
                    pending = []

                    def flush_pv():
                        while pending:
                            kb_, pt_, qs_ = pending.pop(0)
                            for hl_ in range(2):
                                for pr_ in range(2):
                                    h_ = 2 * pr_ + hl_
                                    nc.tensor.matmul(
                                        pv[h_][:, qs_:512], t_vext[:, kb_, :],
                                        pt_[pr_][:, 512 * hl_ + qs_:
                                                 512 * hl_ + 512],
                                        start=(kb_ == 0),
                                        stop=(kb_ == 4 * j + 3))

                    for kb in range(4 * j + 4):
                        diag = kb >= 4 * j
                        qs = 128 * (kb - 4 * j) if diag else 0
                        ksl = slice(128 * kb, 128 * kb + 128)
                        qsl = slice(qlo + qs, qlo + 512)
                        pts = []
                        for pr in range(2):
                            st = ps2.tile([128, 1024], F32, tag="st", bufs=2)
                            for hl in range(2):
                                h = 2 * pr + hl
                                rsl = slice(32 * h, 32 * h + 32)
                                osl = slice(512 * hl + qs, 512 * hl + 512)
                                nc.tensor.matmul(
                                    st[:, osl], t_ku4[rsl, ksl],
                                    t_qhu[rsl, qsl], start=True, stop=False,
                                    tile_position=(32 * h, 0))
                                nc.tensor.matmul(
                                    st[:, osl], t_kl4[rsl, ksl],
                                    t_qhl[rsl, qsl], start=False, stop=True,
                                    tile_position=(32 * h, 0))
                            pt = p2.tile([128, 1024], F32R, tag="pt", bufs=6)
                            if diag:
                                for hl in range(2):
                                    msl = slice(512 * hl + qs,
                                                512 * hl + qs + 128)
                                    nc.vector.tensor_tensor(
                                        out=st[:, msl], in0=st[:, msl],
                                        in1=ct["trimask"][:], op=OP.add)
                                    esl = slice(512 * hl + qs, 512 * hl + 512)
                                    nc.scalar.activation(pt[:, esl],
                                                         st[:, esl], AF.Exp)
                            else:
                                nc.scalar.activation(pt[:], st[:], AF.Exp)
                            pts.append(pt)
                        # PV for the PREVIOUS kb is emitted after this kb's QK,
                        # so PE has ready matmuls while ACT runs this exp.
                        if len(pending) >= 2:
                            kb_, pt_, qs_ = pending.pop(0)
                            for hl_ in range(2):
                                for pr_ in range(2):
                                    h_ = 2 * pr_ + hl_
                                    nc.tensor.matmul(
                                        pv[h_][:, qs_:512], t_vext[:, kb_, :],
                                        pt_[pr_][:, 512 * hl_ + qs_:
                                                 512 * hl_ + 512],
                                        start=(kb_ == 0),
                                        stop=(kb_ == 4 * j + 3))
                        pending.append((kb, pts, qs))
                    flush_pv()
                    for h in range(4):
                        pr, hl = h // 2, h % 2
                        dst = t_yt[pr][64 * hl:64 * hl + 64, qlo:qlo + 512]
                        if hl == 0:
                            nc.vector.tensor_copy(dst, pv[h][0:64, :])
                        else:
                            nc.scalar.copy(dst, pv[h][0:64, :])
                        nc.vector.tensor_copy(t_sumh[h][0:1, qlo:qlo + 512],
                                              pv[h][64:65, :])

            # ---------- Phase 3: normalize + gate, output projection ----------
            with (
                tc.tile_pool(name="p3ps", bufs=1, space="PSUM") as ps3,
                tc.tile_pool(name="p3sb", bufs=1) as p3,
            ):
                t_ow = p3.tile([128, 2, D], F32R, tag="ow")
                nc.sync.dma_start(t_ow[:], dram["ow"][:])
                for h in range(4):
                    nc.sync.dma_start(t_sums[h:h + 1, :], t_sumh[h][:])
                t_rec = p3.tile([4, T], F32, tag="rec")
                t_scr = p3.tile([4, T], F32, tag="scr")
                nc.vector.reciprocal_approx_accurate(t_rec[:], t_sums[:],
                                                     t_scr[:])
                t_sc = p3.tile([4, T], F32R, tag="sc")
                nc.vector.tensor_tensor(out=t_sc[:], in0=t_rec[:],
                                        in1=t_gate[:], op=OP.mult)
                t_yts = [p3.tile([128, T], F32R, tag=f"yts{pr}", name=f"yts{pr}")
                         for pr in range(2)]
                for n in range(NT):
                    sl = slice(512 * n, 512 * n + 512)
                    for pr in range(2):
                        scb_ps = ps3.tile([128, 512], F32, tag="scb", bufs=2)
                        nc.tensor.matmul(scb_ps[:],
                                         ct["bchp"][:, 128 * pr:128 * pr + 128],
                                         t_sc[:, sl], start=True, stop=True)
                        nc.vector.tensor_tensor(out=t_yts[pr][:, sl],
                                                in0=t_yt[pr][:, sl].bitcast(F32),
                                                in1=scb_ps[:], op=OP.mult)
                    for tch in range(4 * n, 4 * n + 4):
                        tsl = slice(128 * tch, 128 * tch + 128)
                        o_ps = ps3.tile([128, D], F32, tag="ops", bufs=2)
                        for e2 in range(2):
                            esl = slice(512 * e2, 512 * e2 + 512)
                            for pr in range(2):
                                nc.tensor.matmul(o_ps[:, esl],
                                                 t_yts[pr][:, tsl],
                                                 t_ow[:, pr, esl],
                                                 start=(pr == 0), stop=(pr == 1))
                        t_o = p3.tile([128, D], F32, tag="osb", bufs=2)
                        if tch % 2 == 0:
                            nc.vector.tensor_copy(t_o[:], o_ps[:])
                        else:
                            nc.scalar.copy(t_o[:], o_ps[:])
                        eng = nc.sync if tch % 2 == 0 else nc.gpsimd
                        eng.dma_start(dram["out"][:, tch, :], t_o[:])


# ======================= host side =======================

def _rope_tables():
    rd = HD
    if T > TSL:
        nb = BASE * (T / TSL) ** (rd / (rd - 2))
    else:
        nb = BASE
    inv = 1.0 / nb ** (np.arange(0, rd, 2, dtype=np.float32) / rd)
    fr = np.arange(T, dtype=np.float32)[:, None] * inv[None, :]
    return (np.ascontiguousarray(np.cos(fr).T.astype(np.float32)),
            np.ascontiguousarray(np.sin(fr).T.astype(np.float32)))


def _const_inputs():
    cosT, sinT = _rope_tables()                             # [32, T]
    cos4 = np.tile(cosT, (4, 1))                            # [128, T]
    sin4 = np.tile(sinT, (4, 1))

    tri = np.where(np.arange(128)[:, None] > np.arange(128)[None, :],
                   np.float32(NEG), np.float32(0.0)).astype(np.float32)
    ident = np.eye(64, dtype=np.float32)

    sumq = np.zeros((128, 5), np.float32)
    for g in range(G):
        sumq[32 * g:32 * g + 32, g] = 1.0
    sumk = np.zeros((64, 5), np.float32)
    sumk[:, 4] = 1.0

    bcq = np.zeros((5, 128), np.float32)
    for m_ in range(128):
        bcq[m_ // 32, m_] = 1.0
    bck = np.zeros((5, 64), np.float32)
    bck[4, :] = 1.0
    bchp = np.zeros((4, 256), np.float32)
    for pr in range(2):
        for m_ in range(128):
            bchp[2 * pr + m_ // 64, 128 * pr + m_] = 1.0

    perm = np.zeros((64, 64), np.float32)
    for m_ in range(H2):
        perm[H2 + m_, m_] = 1.0
    for m_ in range(H2, HD):
        perm[m_ - H2, m_] = -1.0

    vext0 = np.zeros((128, NKB, 65), np.float32)
    vext0[:, :, 64] = 1.0

    return dict(cos4=cos4, sin4=sin4, trimask=tri, ident=ident, sumq=sumq,
                sumk=sumk, bcq=bcq, bck=bck, bchp=bchp, perm=perm, vext0=vext0)


def _prep_core(c, x, qw, kw, vw, ow, ve, v0, q_gain, vr_lambda, gate_w, gate_b,
               const):
    b, h = c // NKV, c % NKV
    lam0 = float(vr_lambda[0])

    xT = np.ascontiguousarray(x[b].T)                       # [D, T]
    xT = xT.reshape(DC, 128, T).transpose(1, 0, 2)          # [128, DC, T]

    qwh = qw.reshape(NH, HD, D)[G * h:G * h + G]            # [4, 64, D]
    qU = qwh[:, 0:H2, :].reshape(128, D)
    qL = qwh[:, H2:HD, :].reshape(128, D)
    qwU = qU.T.reshape(DC, 128, 128).transpose(1, 0, 2)
    qwL = qL.T.reshape(DC, 128, 128).transpose(1, 0, 2)

    kwh = kw.reshape(NKV, HD, D)[h]                         # [64, D]
    vwh = vw.reshape(NKV, HD, D)[h]
    kvr = np.concatenate([kwh[0:H2], kwh[H2:HD], vwh], axis=0)   # [128, D]
    kvw_ = kvr.T.reshape(DC, 128, 128).transpose(1, 0, 2)

    gwh = gate_w[G * h:G * h + G]                           # [4, D]
    gwt = gwh.T.reshape(DC, 128, G).transpose(1, 0, 2)

    ows = ow.T[256 * h:256 * h + 256, :]                    # [256, D]
    owt = ows.reshape(2, 128, D).transpose(1, 0, 2)

    veT = ve[b, :, HD * h:HD * h + HD].T                    # [64, T]
    v0T = v0[b, :, h, :].T
    vaux = np.concatenate([veT, lam0 * v0T], axis=0)        # [128, T]

    gains = q_gain[G * h:G * h + G].astype(np.float64)
    lnsc = np.zeros((5, 1), np.float32)
    lnbi = np.zeros((5, 1), np.float32)
    for g in range(G):
        f = (8.0 / gains[g]) ** 2
        lnsc[g, 0] = f / HD
        lnbi[g, 0] = EPS * f
    lnsc[4, 0] = 1.0 / HD
    lnbi[4, 0] = EPS

    gb = gate_b[G * h:G * h + G].astype(np.float32).reshape(4, 1)

    m = dict(xT=xT, qwU=qwU, qwL=qwL, kvw=kvw_, gw=gwt, ow=owt,
             vaux=vaux, lnsc=lnsc, lnbi=lnbi, gb=gb)
    m.update(const)
    return {k: np.ascontiguousarray(v) for k, v in m.items()}


def kernel(x, qw, kw, vw, ow, ve, v0, q_gain, vr_lambda, gate_w, gate_b,
           _reps=1, _return_res=False):
    x = np.asarray(x, np.float32)
    qw = np.asarray(qw, np.float32)
    kw = np.asarray(kw, np.float32)
    vw = np.asarray(vw, np.float32)
    ow = np.asarray(ow, np.float32)
    ve = np.asarray(ve, np.float32)
    v0 = np.asarray(v0, np.float32)
    q_gain = np.asarray(q_gain, np.float32)
    vr_lambda = np.asarray(vr_lambda, np.float32)
    gate_w = np.asarray(gate_w, np.float32)
    gate_b = np.asarray(gate_b, np.float32)

    lam1 = float(vr_lambda[1])
    key = (lam1, _reps)
    if key not in _CACHE:
        _CACHE[key] = _build_program(lam1, _reps)
    nc = _CACHE[key]

    const = _const_inputs()
    in_maps = [
        _prep_core(c, x, qw, kw, vw, ow, ve, v0, q_gain, vr_lambda,
                   gate_w, gate_b, const)
        for c in range(8)
    ]
    res = run_bass_kernel_spmd(nc, in_maps, core_ids=list(range(8)))

    out = np.zeros((B, T, D), np.float32)
    raw_v = np.zeros((B, T, NKV, HD), np.float32)
    for c in range(8):
        b, h = c // NKV, c % NKV
        po = res.results[c]["out_p"]                        # [128, 16, D]
        out[b] += po.transpose(1, 0, 2).reshape(T, D)
        raw_v[b, :, h, :] = res.results[c]["raw_v"].T
    if _return_res:
        return (out, raw_v), res
    return out, raw_v
